# revision 47
# baseline (speedup 1.0000x reference)
# Distributed GIN (3-layer) + per-graph softmax on 8 TRN2 NeuronCores.
#
# Sharding: nodes split into 8 contiguous ranges of 6250; edges partitioned by
# destination core so segment_sum is local (computed as one-hot matmuls on the
# TensorEngine); per layer the node features are all-gathered (as y = x @ W1,
# exploiting linearity of segment_sum through W1) into replicated DRAM tables
# that each core gathers its edge sources from via dma_gather.  The table is
# split into two halves (a/b by within-shard offset) so each half's AllGather
# overlaps compute and the int16 gather indices stay in range.  Gathers
# round-robin over 4 SWDGE queues so their random-read DMAs overlap.  Layer 0
# needs no gather: the host materializes x[src] directly (input halo
# exchange), and W1 is applied after aggregation.  The "+x_i" self term of GIN
# is folded into the aggregation as an identity-one-hot matmul against a
# node-major strip (which doubles as the staging buffer for the next layer's
# table write), so no transposed-x prepass is needed.  Per-graph softmax via
# global [256]-wide partial sums + AllReduce (segment-max is skipped: logits
# are bounded and softmax is shift-invariant).
import numpy as np
import ml_dtypes

N = 50000
E = 800000
G = 256
DIMS = [128, 128, 64, 32]
BN_EPS = 1e-5
TEMP = 5.0

NCORES = 8
NLOC = N // NCORES            # 6250
NTILE = 49                    # node tiles per core
NPAD = NTILE * 128            # 6272
ASPL = 3072                   # within-shard offset split for a/b table halves
NA = ASPL * NCORES            # 24576 rows in table a
BROWS = NPAD - ASPL           # 3200 rows per core in table b (tile-aligned, padded)
NB = BROWS * NCORES           # 25600 rows in table b
CH = 2                        # dst-tiles per gather chunk
LEAD = 10                     # stream-a gather chunks issued ahead of consumption
LEADB = 3                     # stream-b gather chunks issued ahead of consumption
OHLEAD = 2                    # one-hot build chunks ahead of consumption
CCA_AT = 23                   # issue half-a AllGather once dst-tiles 0..23 done

MLP_IN = [128, 128, 64]
MLP_M = [128, 64, 32]

BF16 = ml_dtypes.bfloat16

_CACHE = {}


def _chunks():
    out = []
    d = 0
    while d < NTILE:
        out.append((d, min(CH, NTILE - d)))
        d += CH
    return out


def _pack_stream(vals, drel, budgets, chunks):
    """vals/drel: per-dst-tile lists. Returns (idx_wrapped[128, .], drel_T[128, .],
    tile offsets per dst-tile)."""
    offs = np.zeros(NTILE + 1, dtype=np.int64)
    np.cumsum(budgets, out=offs[1:])
    tot = int(offs[-1])
    iv = np.zeros(tot * 128, dtype=np.int64)
    dv = np.full(tot * 128, -1.0, dtype=np.float32)
    for d in range(NTILE):
        n = len(vals[d])
        base = int(offs[d]) * 128
        iv[base:base + n] = vals[d]
        dv[base:base + n] = drel[d]
    cols = []
    for d0, csz in chunks:
        v = iv[offs[d0] * 128:offs[d0 + csz] * 128]
        cols.append(v.reshape(-1, 16).T)
    w = np.tile(np.concatenate(cols, axis=1).astype(np.int16), (8, 1))
    d_t = dv.reshape(tot, 128).T.astype(BF16)
    return w, d_t, offs


def _preprocess(x, edge_index, batch):
    src = np.asarray(edge_index[0], dtype=np.int64)
    dst = np.asarray(edge_index[1], dtype=np.int64)
    batch = np.asarray(batch, dtype=np.int64)
    x = np.asarray(x, dtype=np.float32)
    chunks = _chunks()

    owner = src // NLOC
    off = src % NLOC
    s_ab = (off >= ASPL).astype(np.int64)          # 0 = a, 1 = b
    # pair-packed table rows: row k holds [y[k] | y[k+ASPL]] per owner
    row = owner * BROWS + np.where(s_ab == 0, off, off - ASPL)

    core = dst // NLOC
    dtile = (dst % NLOC) // 128
    key = (core * NTILE + dtile) * 2 + s_ab
    order = np.argsort(key, kind="stable")
    srow = row[order]
    sdst = dst[order]
    ssrc = src[order]
    counts = np.bincount(key, minlength=NCORES * NTILE * 2).reshape(NCORES, NTILE, 2)
    starts = np.zeros(NCORES * NTILE * 2 + 1, dtype=np.int64)
    np.cumsum(counts.reshape(-1), out=starts[1:])
    # per-(dst-tile, stream) tile budgets, shared across cores (max)
    B = np.ceil(counts.max(axis=0) / 128).astype(np.int64)      # [NTILE, 2]
    # layer-0 combined stream budgets
    ccounts = counts.sum(axis=2)                                 # [NCORES, NTILE]
    B0 = np.ceil(ccounts.max(axis=0) / 128).astype(np.int64)     # [NTILE]

    per_core = []
    for r in range(NCORES):
        vals = {0: [], 1: []}
        drels = {0: [], 1: []}
        xevals = []
        xdrels = []
        for d in range(NTILE):
            xs = []
            xd = []
            for s in (0, 1):
                k = (r * NTILE + d) * 2 + s
                a, b = starts[k], starts[k + 1]
                vals[s].append(srow[a:b])
                dr = (sdst[a:b] - (r * NLOC + d * 128)).astype(np.float32)
                drels[s].append(dr)
                xs.append(ssrc[a:b])
                xd.append(dr)
            xevals.append(np.concatenate(xs))
            xdrels.append(np.concatenate(xd))
        idx_a, drel_a, offa = _pack_stream(vals[0], drels[0], B[:, 0], chunks)
        idx_b, drel_b, offb = _pack_stream(vals[1], drels[1], B[:, 1], chunks)

        # layer-0 host-materialized edge stream (x[src], edge-major, pre-tiled)
        off0 = np.zeros(NTILE + 1, dtype=np.int64)
        np.cumsum(B0, out=off0[1:])
        tot0 = int(off0[-1])
        xe = np.zeros((tot0 * 128, 128), dtype=np.float32)
        drc = np.full(tot0 * 128, -1.0, dtype=np.float32)
        for d in range(NTILE):
            n = len(xevals[d])
            base = int(off0[d]) * 128
            xe[base:base + n] = x[xevals[d]]
            drc[base:base + n] = xdrels[d]
        xe_t = xe.reshape(tot0, 128, 128).transpose(1, 0, 2).reshape(128, tot0 * 128).astype(BF16)
        drc_t = drc.reshape(tot0, 128).T.astype(BF16)

        bl = batch[r * NLOC:(r + 1) * NLOC].astype(np.float32)
        bpad = np.concatenate([bl, np.full(NPAD - NLOC, -1.0, np.float32)])
        xp = np.zeros((NPAD, 128), np.float32)
        xp[:NLOC] = x[r * NLOC:(r + 1) * NLOC]
        per_core.append(dict(
            x_bf=xp.astype(BF16), x_edges=xe_t, drel_c=drc_t,
            idx_a=idx_a, idx_b=idx_b, drel_a=drel_a, drel_b=drel_b,
            brow=np.tile(bpad, (128, 1)).astype(BF16),
            batchT=bpad.reshape(NTILE, 128).T.astype(BF16),
        ))
    shape_key = (tuple(B[:, 0]), tuple(B[:, 1]), tuple(B0))
    return per_core, shape_key


def _weights(inputs):
    w = {}
    for l in range(3):
        w[f"w1_{l}"] = np.ascontiguousarray(np.asarray(inputs[f"W1_{l}"], np.float32)).astype(BF16)
        w[f"w2_{l}"] = np.ascontiguousarray(np.asarray(inputs[f"W2_{l}"], np.float32)).astype(BF16)
    w["wlin"] = (np.asarray(inputs["W_lin"], np.float32) / TEMP).astype(BF16)
    vec = np.zeros((128, 11), np.float32)
    for l in range(3):
        m = MLP_M[l]
        g = np.asarray(inputs[f"gamma_{l}"], np.float32)
        be = np.asarray(inputs[f"beta_{l}"], np.float32)
        mu = np.asarray(inputs[f"mean_{l}"], np.float32)
        va = np.asarray(inputs[f"var_{l}"], np.float32)
        b1 = np.asarray(inputs[f"b1_{l}"], np.float32)
        b2 = np.asarray(inputs[f"b2_{l}"], np.float32)
        scale = g / np.sqrt(va + BN_EPS)
        shift = be - mu * scale + b2 * scale
        vec[:m, 3 * l + 0] = b1
        vec[:m, 3 * l + 1] = scale
        vec[:m, 3 * l + 2] = shift
    vec[:, 9] = np.arange(128, dtype=np.float32)
    vec[:, 10] = np.arange(128, dtype=np.float32) + 128.0
    w["vec"] = vec
    w["blin_t"] = float(np.asarray(inputs["b_lin"], np.float32).reshape(-1)[0]) / TEMP
    ar = np.arange(128, dtype=np.float32)
    w["iota_e"] = np.tile(ar, (128, 1)).astype(BF16)
    w["iota_g0"] = np.tile(ar, (128, 1)).astype(BF16)
    w["iota_g1"] = (np.tile(ar, (128, 1)) + 128.0).astype(BF16)
    w["ident_b"] = np.eye(128, dtype=np.float32).astype(BF16)
    return w


def _build(shape_key, blin_t, stage=4):
    import concourse.bacc as bacc
    import concourse.tile as tile
    from concourse import mybir

    f32 = mybir.dt.float32
    bf16 = mybir.dt.bfloat16
    i16 = mybir.dt.int16
    RELU = mybir.ActivationFunctionType.Relu
    IDENT = mybir.ActivationFunctionType.Identity
    EXP = mybir.ActivationFunctionType.Exp
    EQ = mybir.AluOpType.is_equal
    ADD = mybir.AluOpType.add

    Ba = np.array(shape_key[0], dtype=np.int64)
    Bb = np.array(shape_key[1], dtype=np.int64)
    B0 = np.array(shape_key[2], dtype=np.int64)
    offa = np.zeros(NTILE + 1, np.int64); np.cumsum(Ba, out=offa[1:])
    offb = np.zeros(NTILE + 1, np.int64); np.cumsum(Bb, out=offb[1:])
    off0 = np.zeros(NTILE + 1, np.int64); np.cumsum(B0, out=off0[1:])
    TOTA, TOTB, TOT0 = int(offa[-1]), int(offb[-1]), int(off0[-1])
    chunks = _chunks()
    maxnt = {"a": max(int(offa[d0 + c] - offa[d0]) for d0, c in chunks),
             "b": max(int(offb[d0 + c] - offb[d0]) for d0, c in chunks),
             "c": max(int(off0[d0 + c] - off0[d0]) for d0, c in
                      [(d, min(2, NTILE - d)) for d in range(0, NTILE, 2)])}
    gmax = max(maxnt.values())

    nc = bacc.Bacc("TRN2", target_bir_lowering=False, debug=False,
                   num_devices=NCORES, num_swdge_queues=4,
                   dynamic_dma_scratch_size=32768)

    x_in = nc.dram_tensor("x_bf", [NPAD, 128], bf16, kind="ExternalInput")
    xe_in = nc.dram_tensor("x_edges", [128, TOT0 * 128], bf16, kind="ExternalInput")
    drelc_in = nc.dram_tensor("drel_c", [128, TOT0], bf16, kind="ExternalInput")
    idx_in = {"a": nc.dram_tensor("idx_a", [128, TOTA * 8], i16, kind="ExternalInput"),
              "b": nc.dram_tensor("idx_b", [128, TOTB * 8], i16, kind="ExternalInput")}
    drel_in = {"a": nc.dram_tensor("drel_a", [128, TOTA], bf16, kind="ExternalInput"),
               "b": nc.dram_tensor("drel_b", [128, TOTB], bf16, kind="ExternalInput")}
    brow_in = nc.dram_tensor("brow", [128, NPAD], bf16, kind="ExternalInput")
    batchT_in = nc.dram_tensor("batchT", [128, NTILE], bf16, kind="ExternalInput")
    w1_in = [nc.dram_tensor(f"w1_{l}", [MLP_IN[l], MLP_M[l]], bf16, kind="ExternalInput") for l in range(3)]
    w2_in = [nc.dram_tensor(f"w2_{l}", [MLP_M[l], MLP_M[l]], bf16, kind="ExternalInput") for l in range(3)]
    wlin_in = nc.dram_tensor("wlin", [32, 1], bf16, kind="ExternalInput")
    vec_in = nc.dram_tensor("vec", [128, 11], f32, kind="ExternalInput")
    iota_e_in = nc.dram_tensor("iota_e", [128, 128], bf16, kind="ExternalInput")
    iota_g0_in = nc.dram_tensor("iota_g0", [128, 128], bf16, kind="ExternalInput")
    iota_g1_in = nc.dram_tensor("iota_g1", [128, 128], bf16, kind="ExternalInput")
    ident_b_in = nc.dram_tensor("ident_b", [128, 128], bf16, kind="ExternalInput")
    out_dram = nc.dram_tensor("out", [NPAD, 1], f32, kind="ExternalOutput")
    dbg_dram = nc.dram_tensor("dbg", [128, NPAD], f32, kind="ExternalOutput") if stage < 4 else None

    rg = [list(range(NCORES))]

    with tile.TileContext(nc) as tc:
        with (
            tc.tile_pool(name="persist", bufs=1) as pp,
            tc.tile_pool(name="dram", bufs=1, space="DRAM") as dp,
            tc.tile_pool(name="work", bufs=3) as wp,
            tc.tile_pool(name="psA", bufs=2, space="PSUM") as psA,
            tc.tile_pool(name="psB", bufs=6, space="PSUM") as psB,
        ):
            idx_sb = {}
            drel_sb = {}
            for s in ("a", "b"):
                wdt = TOTA if s == "a" else TOTB
                idx_sb[s] = pp.tile([128, wdt * 8], i16, tag=f"idx_{s}", name=f"idx_{s}")
                nc.sync.dma_start(out=idx_sb[s][:], in_=idx_in[s][:])
                drel_sb[s] = pp.tile([128, wdt], bf16, tag=f"drel_{s}", name=f"drel_{s}")
                nc.sync.dma_start(out=drel_sb[s][:], in_=drel_in[s][:])
            drel_c = pp.tile([128, TOT0], bf16, tag="drel_c", name="drel_c")
            nc.sync.dma_start(out=drel_c[:], in_=drelc_in[:])
            batchT = pp.tile([128, NTILE], bf16, tag="batchT", name="batchT")
            nc.sync.dma_start(out=batchT[:], in_=batchT_in[:])
            w1 = []
            w2 = []
            for l in range(3):
                t1_ = pp.tile([MLP_IN[l], MLP_M[l]], bf16, tag=f"w1_{l}", name=f"w1s_{l}")
                nc.sync.dma_start(out=t1_[:], in_=w1_in[l][:])
                w1.append(t1_)
                t2_ = pp.tile([MLP_M[l], MLP_M[l]], bf16, tag=f"w2_{l}", name=f"w2s_{l}")
                nc.sync.dma_start(out=t2_[:], in_=w2_in[l][:])
                w2.append(t2_)
            wlin = pp.tile([32, 1], bf16, tag="wlin", name="wlin")
            nc.sync.dma_start(out=wlin[:], in_=wlin_in[:])
            vec = pp.tile([128, 11], f32, tag="vec", name="vec")
            nc.sync.dma_start(out=vec[:], in_=vec_in[:])
            iota_e = pp.tile([128, 128], bf16, tag="iota_e", name="iota_e")
            nc.sync.dma_start(out=iota_e[:], in_=iota_e_in[:])
            ident_b = pp.tile([128, 128], bf16, tag="ident_b", name="ident_b")
            nc.sync.dma_start(out=ident_b[:], in_=ident_b_in[:])

            # node-major y strips: self-edge source for layer l (strip[l%2]),
            # written during layer l-1 (doubles as DMA staging for yloc).
            strip = [pp.tile([128, NPAD], bf16, tag=f"strip{i}", name=f"strip{i}") for i in range(2)]
            nc.sync.dma_start(
                out=strip[0][:].rearrange("p (i e) -> p i e", e=128),
                in_=x_in[:].rearrange("(i p) e -> p i e", p=128),
            )
            e_strip = pp.tile([128, NTILE], f32, tag="e_strip", name="e_strip")
            out_strip = pp.tile([128, NTILE], f32, tag="out_strip", name="out_strip")
            r_str = pp.tile([128, NTILE], f32, tag="r_str", name="r_str")

            # pair-packed y table: row k of a layer's table holds
            # [y[k] (w cols) | y[k+ASPL] (w cols)], w = MLP_M[l]; halves the
            # AllGather payload and needs one collective per layer.
            ATIL = ASPL // 128   # 24 tiles in the a-range
            yloc = [None] + [dp.tile([BROWS, 128], bf16, tag=f"yloc{l}", name=f"yloc{l}") for l in (1, 2)]
            tf = [None] + [dp.tile([NB, 128], bf16, tag=f"tf{l}", name=f"tf{l}") for l in (1, 2)]
            ag_in = dp.tile([128, 2], f32, tag="ag_in", name="ag_in")
            ag_out = dp.tile([NCORES * 128, 2], f32, tag="ag_out", name="ag_out")

            def dma_rows(l, r0, nrow_t, src_sb):
                w = MLP_M[l]
                if r0 < ATIL:
                    seg = yloc[l][r0 * 128:(r0 + nrow_t) * 128, 0:w]
                else:
                    seg = yloc[l][(r0 - ATIL) * 128:(r0 - ATIL + nrow_t) * 128, w:2 * w]
                nc.sync.dma_start(
                    out=seg.rearrange("(i p) e -> p i e", p=128),
                    in_=src_sb.rearrange("p (i e) -> p i e", e=128)[:, :, 0:w],
                )

            def cc_full(l):
                nc.gpsimd.collective_compute(
                    "AllGather", mybir.AluOpType.bypass, replica_groups=rg,
                    ins=[yloc[l][:]], outs=[tf[l][:]])

            def build_oh(dst_tile, drl_src, c0, nt):
                drl = drl_src[:, c0:c0 + nt]
                nc.vector.tensor_tensor(
                    out=dst_tile[:, :nt, :],
                    in0=drl.rearrange("p (t o) -> p t o", o=1).to_broadcast([128, nt, 128]),
                    in1=iota_e[:].rearrange("p (o e) -> p o e", o=1).to_broadcast([128, nt, 128]),
                    op=EQ)

            # ================= layers =================
            def epilogue(l, t, ps_agg):
                m = MLP_M[l]
                m2 = MLP_M[l + 1] if l < 2 else None
                last = l == 2
                if l == 0:
                    t1 = wp.tile([128, 128], bf16, tag="t1", name="t1")
                    nc.scalar.copy(out=t1[:], in_=ps_agg[:])
                    ps_i = psB.tile([128, 128], f32, tag="ps_tmp", name="ps_i")
                    nc.tensor.matmul(out=ps_i[:m, :], lhsT=w1[0][:], rhs=t1[:, :],
                                     start=True, stop=True)
                    hin = ps_i
                else:
                    hin = ps_agg
                h = wp.tile([128, 128], bf16, tag="h", name="h")
                nc.scalar.activation(h[:m, :], hin[:m, :], RELU,
                                     bias=vec[:m, 3 * l:3 * l + 1], scale=1.0)
                ps_z = psB.tile([128, 128], f32, tag="ps_tmp", name="ps_z")
                nc.tensor.matmul(out=ps_z[:m, :], lhsT=w2[l][:], rhs=h[:m, :],
                                 start=True, stop=True)
                xn = wp.tile([128, 128], bf16, tag="xn", name="xn")
                nc.scalar.activation(xn[:m, :], ps_z[:m, :],
                                     IDENT if last else RELU,
                                     bias=vec[:m, 3 * l + 2:3 * l + 3],
                                     scale=vec[:m, 3 * l + 1:3 * l + 2])
                if not last:
                    ps_y = psB.tile([128, 128], f32, tag="ps_tmp", name="ps_yn")
                    nc.tensor.matmul(out=ps_y[:m2, :], lhsT=w1[l + 1][:],
                                     rhs=xn[:m, :], start=True, stop=True)
                    ty = wp.tile([128, 128], bf16, tag="ty", name="ty")
                    nc.scalar.copy(out=ty[:m2, :], in_=ps_y[:m2, :])
                    ps_t = psB.tile([128, 128], bf16, tag="ps_tmp", name="ps_t2")
                    nc.tensor.transpose(out=ps_t[:], in_=ty[:, :], identity=ident_b[:])
                    nc.scalar.copy(out=strip[(l + 1) % 2][:, t * 128:(t + 1) * 128], in_=ps_t[:])
                else:
                    ps_lg = psB.tile([128, 1], f32, tag="ps_tmp", name="ps_lg")
                    nc.tensor.matmul(out=ps_lg[:], lhsT=xn[:m, :], rhs=wlin[:],
                                     start=True, stop=True)
                    nc.scalar.activation(e_strip[:, t:t + 1], ps_lg[:], EXP,
                                         bias=blin_t, scale=1.0)

            # ---------- layer 0 (host-materialized edge stream) ----------
            with (
                tc.tile_pool(name="gx", bufs=3) as gpx,
                tc.tile_pool(name="gox", bufs=2) as gox,
            ):
                l0chunks = [(d, min(2, NTILE - d)) for d in range(0, NTILE, 2)]
                for d0, csz in l0chunks:
                    nt = int(off0[d0 + csz] - off0[d0])
                    xe = gpx.tile([128, maxnt["c"], 128], bf16, tag="g_x", name="xe")
                    nc.sync.dma_start(
                        out=xe[:, :nt, :].rearrange("p t e -> p (t e)"),
                        in_=xe_in[:, off0[d0] * 128:off0[d0 + csz] * 128])
                    oh0 = gox.tile([128, maxnt["c"], 128], bf16, tag="oh_x", name="oh0")
                    build_oh(oh0, drel_c, int(off0[d0]), nt)
                    for i in range(csz):
                        t = d0 + i
                        ps_agg = psA.tile([128, 128], f32, tag="ps_agg", name="ps_agg")
                        nb_ = int(B0[t])
                        nc.tensor.matmul(
                            out=ps_agg[:], lhsT=strip[0][:, t * 128:(t + 1) * 128],
                            rhs=ident_b[:], start=True, stop=(nb_ == 0))
                        base = int(off0[t] - off0[d0])
                        for j in range(nb_):
                            nc.tensor.matmul(
                                out=ps_agg[:], lhsT=xe[:, base + j, :], rhs=oh0[:, base + j, :],
                                start=False, stop=(j == nb_ - 1))
                        epilogue(0, t, ps_agg)
                    dma_rows(1, d0, csz, strip[1][:, d0 * 128:(d0 + csz) * 128])
                cc_full(1)

            # ---------- layers 1, 2 (table gathers) ----------
            # Deep a-stream lookahead (gathers only; one-hots are built by the
            # vector engine a couple of chunks ahead of consumption so the deep
            # pipeline only pays for the gathered-data buffers).
            with (
                tc.tile_pool(name="ga", bufs=LEAD + 1) as gpa,
                tc.tile_pool(name="goa", bufs=OHLEAD + 1) as goa,
                tc.tile_pool(name="gb", bufs=LEADB + 1) as gpb,
            ):
                for l in (1, 2):
                    last = l == 2
                    nch = len(chunks)
                    g_store = {}
                    oh_store = {}
                    qrr = [l]

                    def issue_g(ci, s):
                        d0, csz = chunks[ci]
                        offs = offa if s == "a" else offb
                        pool = gpa if s == "a" else gpb
                        tfx = tf[l]
                        nt = int(offs[d0 + csz] - offs[d0])
                        g = pool.tile([128, maxnt[s], 128], bf16, tag=f"g_{s}", name=f"g_{s}")
                        nc.gpsimd.dma_gather(
                            g[:, :nt, :], tfx[:],
                            idx_sb[s][:, int(offs[d0]) * 8:int(offs[d0 + csz]) * 8],
                            num_idxs=nt * 128, num_idxs_reg=nt * 128, elem_size=128,
                            single_packet=False,
                            queue_num=qrr[0] % 4,
                        )
                        qrr[0] += 1
                        g_store[(ci, s)] = g

                    def build_oh_for(ci, s):
                        d0, csz = chunks[ci]
                        offs = offa if s == "a" else offb
                        nt = int(offs[d0 + csz] - offs[d0])
                        oh = goa.tile([128, maxnt[s], 128], bf16, tag=f"oh_{s}", name=f"oh_{s}")
                        build_oh(oh, drel_sb[s], int(offs[d0]), nt)
                        oh_store[(ci, s)] = oh

                    def process_chunk(ci):
                        d0, csz = chunks[ci]
                        for i in range(csz):
                            t = d0 + i
                            ps_agg = psA.tile([128, 128], f32, tag="ps_agg", name="ps_agg")
                            ntt = int(Ba[t] + Bb[t])
                            w = MLP_M[l]
                            nc.tensor.matmul(
                                out=ps_agg[:w, :], lhsT=strip[l % 2][:, t * 128:t * 128 + w],
                                rhs=ident_b[:], start=True, stop=(ntt == 0))
                            k = 0
                            for s, offs, c0 in (("a", offa, 0), ("b", offb, w)):
                                nb_ = int((Ba if s == "a" else Bb)[t])
                                base = int(offs[t] - offs[d0])
                                g = g_store[(ci, s)]
                                oh = oh_store[(ci, s)]
                                for j in range(nb_):
                                    nc.tensor.matmul(
                                        out=ps_agg[:w, :], lhsT=g[:, base + j, c0:c0 + w],
                                        rhs=oh[:, base + j, :],
                                        start=False, stop=(k == ntt - 1))
                                    k += 1
                            epilogue(l, t, ps_agg)
                        if not last:
                            dma_rows(l + 1, d0, csz,
                                     strip[(l + 1) % 2][:, d0 * 128:(d0 + csz) * 128])

                    for ci in range(min(LEAD, nch)):
                        issue_g(ci, "a")
                    for ci in range(min(LEADB, nch)):
                        issue_g(ci, "b")
                    for ci in range(min(OHLEAD, nch)):
                        build_oh_for(ci, "a")
                        build_oh_for(ci, "b")
                    for ci in range(nch):
                        if ci + LEADB < nch:
                            issue_g(ci + LEADB, "b")
                        if ci + LEAD < nch:
                            issue_g(ci + LEAD, "a")
                        if ci + OHLEAD < nch:
                            build_oh_for(ci + OHLEAD, "a")
                            build_oh_for(ci + OHLEAD, "b")
                        process_chunk(ci)
                    if not last:
                        cc_full(l + 1)

            # ================= per-graph softmax =================
            if True:
                with tc.tile_pool(name="tail", bufs=1) as tp:
                    brow = tp.tile([128, NPAD], bf16, tag="brow", name="brow")
                    nc.sync.dma_start(out=brow[:], in_=brow_in[:])
                    iota_g0 = tp.tile([128, 128], bf16, tag="iota_g0", name="iota_g0")
                    nc.sync.dma_start(out=iota_g0[:], in_=iota_g0_in[:])
                    iota_g1 = tp.tile([128, 128], bf16, tag="iota_g1", name="iota_g1")
                    nc.sync.dma_start(out=iota_g1[:], in_=iota_g1_in[:])
                    e_b = tp.tile([128, NTILE], bf16, tag="e_b", name="e_b")
                    sbt = tp.tile([128, NTILE * 128], bf16, tag="sbt", name="sbt")
                    sbt2 = tp.tile([128, NTILE * 128], bf16, tag="sbt2", name="sbt2")
                    sb0 = tp.tile([128, NTILE * 128], bf16, tag="sb0", name="sb0")
                    sb1 = tp.tile([128, NTILE * 128], bf16, tag="sb1", name="sb1")

                    nc.vector.tensor_copy(out=e_b[:], in_=e_strip[:])
                    ps_g0 = psA.tile([128, 1], f32, tag="ps_agg", name="ps_g0")
                    ps_g1 = psA.tile([128, 1], f32, tag="ps_agg", name="ps_g1")
                    gsum = pp.tile([128, 2], f32, tag="gsum", name="gsum")
                    for h_, iog in ((0, iota_g0), (1, iota_g1)):
                        sb_h = sbt if h_ == 0 else sbt2
                        nc.vector.tensor_tensor(
                            out=sb_h[:].rearrange("p (t e) -> p t e", e=128),
                            in0=batchT[:].rearrange("p (t o) -> p t o", o=1).to_broadcast([128, NTILE, 128]),
                            in1=iog[:].rearrange("p (o e) -> p o e", o=1).to_broadcast([128, NTILE, 128]),
                            op=EQ)
                        ps = ps_g0 if h_ == 0 else ps_g1
                        for t in range(NTILE):
                            nc.tensor.matmul(out=ps[:], lhsT=sb_h[:, t * 128:(t + 1) * 128],
                                             rhs=e_b[:, t:t + 1],
                                             start=(t == 0), stop=(t == NTILE - 1))
                        nc.scalar.copy(out=gsum[:, h_:h_ + 1], in_=ps[:])
                    nc.sync.dma_start(out=ag_in[:], in_=gsum[:])
                    nc.gpsimd.collective_compute(
                        "AllGather", mybir.AluOpType.bypass, replica_groups=rg,
                        ins=[ag_in[:]], outs=[ag_out[:]])
                    # overlap the collective with the one-hot builds for the final pass
                    for h_, sb in ((0, sb0), (1, sb1)):
                        nc.vector.tensor_scalar(sb[:], brow[:], vec[:, 9 + h_:10 + h_], None, EQ)
                    s_all = pp.tile([128, 2 * NCORES], f32, tag="s_all", name="s_all")
                    nc.sync.dma_start(
                        out=s_all[:].rearrange("p (r c) -> p r c", c=2),
                        in_=ag_out[:].rearrange("(r p) c -> p r c", p=128))
                    s_red = pp.tile([128, 2], f32, tag="s_red", name="s_red")
                    nc.vector.tensor_tensor(out=s_red[:], in0=s_all[:, 0:2], in1=s_all[:, 2:4], op=ADD)
                    for r in range(2, NCORES):
                        nc.vector.tensor_tensor(out=s_red[:], in0=s_red[:],
                                                in1=s_all[:, 2 * r:2 * r + 2], op=ADD)
                    r_all = pp.tile([128, 2], f32, tag="r_all", name="r_all")
                    nc.vector.reciprocal(out=r_all[:], in_=s_red[:])
                    r_b = pp.tile([128, 2], bf16, tag="r_b", name="r_b")
                    nc.vector.tensor_copy(out=r_b[:], in_=r_all[:])
                    for h_, sb in ((0, sb0), (1, sb1)):
                        for t in range(NTILE):
                            ps_r = psB.tile([128, 1], f32, tag="ps_tmp", name="ps_r")
                            nc.tensor.matmul(out=ps_r[:], lhsT=sb[:, t * 128:(t + 1) * 128],
                                             rhs=r_b[:, h_:h_ + 1], start=True, stop=True)
                            if h_ == 0:
                                nc.scalar.copy(out=r_str[:, t:t + 1], in_=ps_r[:])
                            else:
                                tmp = wp.tile([128, 1], f32, tag="tmp_r", name="tmp_r")
                                nc.vector.tensor_tensor(out=tmp[:], in0=r_str[:, t:t + 1],
                                                        in1=ps_r[:], op=ADD)
                                nc.vector.tensor_tensor(out=out_strip[:, t:t + 1],
                                                        in0=e_strip[:, t:t + 1], in1=tmp[:],
                                                        op=mybir.AluOpType.mult)
                    nc.sync.dma_start(
                        out=out_dram[:].rearrange("(t p) one -> p (t one)", p=128),
                        in_=out_strip[:])

    nc.compile()
    return nc


def _in_maps(per_core, w):
    shared = dict(
        wlin=w["wlin"], vec=w["vec"], iota_e=w["iota_e"],
        iota_g0=w["iota_g0"], iota_g1=w["iota_g1"],
        ident_b=w["ident_b"],
    )
    for l in range(3):
        shared[f"w1_{l}"] = w[f"w1_{l}"]
        shared[f"w2_{l}"] = w[f"w2_{l}"]
    maps = []
    for r in range(NCORES):
        pc = per_core[r]
        maps.append(dict(
            x_bf=pc["x_bf"], x_edges=pc["x_edges"], drel_c=pc["drel_c"],
            idx_a=pc["idx_a"], idx_b=pc["idx_b"],
            drel_a=pc["drel_a"], drel_b=pc["drel_b"],
            brow=pc["brow"], batchT=pc["batchT"], **shared,
        ))
    return maps


def kernel(**inputs):
    import time
    from concourse.bass_utils import run_bass_kernel_spmd

    per_core, shape_key = _preprocess(inputs["x"], inputs["edge_index"], inputs["batch"])
    w = _weights(inputs)

    key = ("v12", shape_key)
    if key not in _CACHE:
        _CACHE[key] = _build(shape_key, w["blin_t"])
    nc = _CACHE[key]

    maps = _in_maps(per_core, w)
    last = None
    for attempt in range(3):
        try:
            res = run_bass_kernel_spmd(nc, maps, list(range(NCORES)))
            break
        except Exception as e:   # wedged device from a prior crash: retry
            last = e
            time.sleep(20)
    else:
        raise last
    out = np.concatenate([res.results[r]["out"][:NLOC] for r in range(NCORES)], axis=0)
    return out.astype(np.float32)
